# revision 39
# baseline (speedup 1.0000x reference)
"""Trainium2 Bass kernel for batched 8-head local-window attention.

Shapes (hardcoded): x [32, 512, 512], w_qkv [512, 1536], w_proj [512, 512],
b_proj [512], mask [1, 1, 512, 512] additive (0 or -1e30).

Strategy: data-parallel over batch across 8 cores (4 batch elements each).
All matmuls in bf16 (fp32 PSUM accumulation). Layouts chosen so that no
input transposes are needed on device:
  - host supplies xT [C, N] per batch
  - qT,kT computed channel-major ([ch, n]) with w_qkv as stationary
  - v computed token-major ([n, ch]) with xT chunks as stationary
  - S^T = K @ Q^T per head ([m, n], key-major, two heads row-packed in the
    PE array) so softmax sums arrive via a ones-column in the attn@V matmul
  - attn@V uses masked exp(S^T) full [128,128] chunks as stationary and
    [v | 1] as moving; normalization is one reciprocal + one broadcast
    multiply per query block
  - out head-concat is PE-transposed to channel-major for the projection
Mask is applied as a 0/1 multiply after exp (exp never sees -1e30; scores
are O(10) so no max-subtraction is needed). Block-level structure (which
128x128 chunks are entirely masked) is derived from the actual mask argument
at call time, so a dense (all-zero) mask also works.

The batch loop is software-pipelined at instruction-emission order (engines
execute their streams in order): batch b+1's qkv and score matmuls are
interleaved with batch b's attnV blocks so the PE never waits on the
scalar-engine exp chain and the HAM clock stays warm.
"""

import numpy as np
import ml_dtypes

B, N, C = 32, 512, 512
HEADS = 8
HD = C // HEADS
SCALE = HD ** -0.5
NCORES = 8
BPC = B // NCORES  # batches per core
P = 128            # partitions
NT = N // P        # 4 n/m tiles of 128
CT = C // P        # 4 channel tiles of 128

_BF16 = ml_dtypes.bfloat16

_cache = {}


def _mask_structure(mask2d):
    """Derive block structure from the additive mask [n, m]."""
    vis = mask2d == 0.0  # [n, m] True = visible
    assert vis.any(axis=1).all(), "some query attends to nothing"
    # Per key-tile t: storage window [offs, offs+W) is 128-block aligned so
    # every attn@V chunk is a full [128,128] stationary; exp only writes the
    # true visible sub-window [offs+elo, offs+elo+width); the rest of the
    # window that chunks can read ("pads") is memset to zero.
    offs, elos, widths, spans = [], [], [], []
    for t in range(NT):
        sub = vis[:, t * P:(t + 1) * P]  # [n, 128]
        rows = np.nonzero(sub.any(axis=1))[0]
        vlo, vhi = int(rows.min()), int(rows.max()) + 1
        o = (vlo // P) * P
        span = ((vhi + P - 1) // P) * P - o
        offs.append(o)
        elos.append(vlo - o)
        widths.append(vhi - vlo)
        spans.append(span)
    W = max(spans)  # storage pitch (multiple of 128)
    pads = []  # (t, start_col, width) regions read by chunks but not written
    for t in range(NT):
        if elos[t] > 0:
            pads.append((t, 0, elos[t]))
        end = elos[t] + widths[t]
        if end < spans[t]:
            pads.append((t, end, spans[t] - end))
    chunks = []
    for s in range(NT):
        cl = []
        for t in range(NT):
            blk = vis[s * P:(s + 1) * P, t * P:(t + 1) * P]
            if not blk.any():
                continue
            lo, hi = s * P, (s + 1) * P
            assert lo >= offs[t] and hi <= offs[t] + spans[t]
            cl.append((t, lo, hi))
        assert cl, f"query block {s} has no visible key chunks"
        chunks.append(cl)
    return W, offs, elos, widths, pads, chunks


def _uniform_groups(entries):
    """Group (t, start, width) entries into runs with equal width and a
    uniform (t, start) stride, so each run is one strided AP op."""
    groups = []
    by_w = {}
    for e in entries:
        by_w.setdefault(e[2], []).append(e)
    for w, es in sorted(by_w.items()):
        es = sorted(es)
        while es:
            run = [es[0]]
            for e in es[1:]:
                if len(run) == 1:
                    run.append(e)
                else:
                    d_t = run[1][0] - run[0][0]
                    d_s = run[1][1] - run[0][1]
                    if e[0] - run[-1][0] == d_t and e[1] - run[-1][1] == d_s:
                        run.append(e)
            es = [e for e in es if e not in run]
            groups.append((w, run))
    return groups


def _build(W, offs, elos, widths, pads, chunks):
    import concourse.bass as bass
    import concourse.tile as tile
    import concourse.mybir as mybir
    from concourse import bacc
    from concourse.masks import make_identity

    fp32 = mybir.dt.float32
    bf16 = mybir.dt.bfloat16
    AF = mybir.ActivationFunctionType

    nc = bacc.Bacc("TRN2", target_bir_lowering=False, debug=False)

    d_xt = nc.dram_tensor("xt", [BPC, C, N], bf16, kind="ExternalInput")
    d_wqkv = nc.dram_tensor("wqkv", [C, 3 * C], bf16, kind="ExternalInput")
    d_wproj = nc.dram_tensor("wproj", [C, C], bf16, kind="ExternalInput")
    d_brep = nc.dram_tensor("brep", [P, C], fp32, kind="ExternalInput")
    d_m01 = nc.dram_tensor("m01", [P, NT, W], bf16, kind="ExternalInput")
    d_y = nc.dram_tensor("y", [BPC, N, C], fp32, kind="ExternalOutput")

    mask_groups = _uniform_groups(
        [(t, elos[t], widths[t]) for t in range(NT)])
    pad_groups = _uniform_groups(pads)

    with tile.TileContext(nc) as tc:
        with (
            tc.tile_pool(name="singles", bufs=1) as singles,
            tc.tile_pool(name="xt", bufs=3) as xt_pool,
            tc.tile_pool(name="qk", bufs=2) as qk_pool,
            tc.tile_pool(name="vplus", bufs=2) as v_pool,
            tc.tile_pool(name="apair", bufs=9) as a_pool,
            tc.tile_pool(name="oc", bufs=2) as oc_pool,
            tc.tile_pool(name="rec", bufs=4) as rec_pool,
            tc.tile_pool(name="psS", bufs=2, space="PSUM") as psS_pool,
            tc.tile_pool(name="psB", bufs=2, space="PSUM") as psB_pool,
            tc.tile_pool(name="psO", bufs=2, space="PSUM") as psO_pool,
        ):
            def xt_load(b):
                """Four per-c-tile DMAs so matmuls can start per chunk."""
                xts = []
                for ct in range(CT):
                    x1 = xt_pool.tile([P, N], bf16, tag=f"xt{ct}")
                    nc.sync.dma_start(
                        out=x1, in_=d_xt.ap()[b, ct * P:(ct + 1) * P, :])
                    xts.append(x1)
                return xts

            xts = xt_load(0)
            wq = []
            wq_src = d_wqkv.ap().rearrange("(t p) o -> p t o", p=P)
            for ct in range(CT):
                w1 = singles.tile([P, 3 * C], bf16, tag=f"wqkv{ct}")
                nc.sync.dma_start(out=w1, in_=wq_src[:, ct, :])
                wq.append(w1)

            def load_rest():
                wproj = singles.tile([P, CT, C], bf16)
                nc.sync.dma_start(
                    out=wproj,
                    in_=d_wproj.ap().rearrange("(t p) o -> p t o", p=P))
                m01 = singles.tile([P, NT, W], bf16)
                nc.sync.dma_start(out=m01, in_=d_m01.ap())
                brep = singles.tile([P, C], fp32)
                nc.sync.dma_start(out=brep, in_=d_brep.ap())
                ident = singles.tile([P, P], bf16)
                make_identity(nc, ident)
                return wproj, m01, brep, ident

            def group_ap(base3d, run, w, lead=None):
                """AP over [P, (2,) len(run), w] from a [P, NT, W] view;
                `run` is [(t, start), ...] with uniform stride. With
                lead=(stride, count), adds a leading free dim (head dim)."""
                t0, s0 = run[0][0], run[0][1]
                a = base3d[:, t0, s0:s0 + w]
                step = ((run[1][0] - t0) * W + run[1][1] - s0) \
                    if len(run) > 1 else 1
                dims = [a.ap[0]]
                if lead is not None:
                    dims.append(list(lead))
                dims += [[step, len(run)], [1, w]]
                return bass.AP(tensor=a.tensor, offset=a.offset, ap=dims)

            def qkv_compute(xts):
                """qT/kT (channel-major) and v+ones (token-major)."""
                qk = qk_pool.tile([P, 2 * CT, N], bf16, tag="qk")
                for jj in range(2 * CT):
                    ps = psB_pool.tile([P, N], fp32, tag="psB")
                    for ct in range(CT):
                        nc.tensor.matmul(
                            ps,
                            lhsT=wq[ct][:, jj * P:(jj + 1) * P],
                            rhs=xts[ct],
                            start=(ct == 0), stop=(ct == CT - 1))
                    if jj % 2 == 0:
                        nc.vector.tensor_copy(out=qk[:, jj, :], in_=ps)
                    else:
                        nc.scalar.copy(out=qk[:, jj, :], in_=ps)
                vplus = v_pool.tile([P, NT, HEADS, HD + 1], bf16, tag="vplus")
                for t in range(NT):
                    ps = psB_pool.tile([P, C], fp32, tag="psB")
                    for ct in range(CT):
                        nc.tensor.matmul(
                            ps,
                            lhsT=xts[ct][:, t * P:(t + 1) * P],
                            rhs=wq[ct][:, 2 * C:3 * C],
                            start=(ct == 0), stop=(ct == CT - 1))
                    nc.vector.tensor_copy(
                        out=vplus[:, t, :, 0:HD],
                        in_=ps.rearrange("p (h d) -> p h d", h=HEADS))
                nc.vector.memset(vplus[:, :, :, HD:HD + 1], 1.0)
                return qk, vplus

            def new_apair():
                apair = a_pool.tile([P, 2, NT, W], bf16, tag="apair")
                for w, run in pad_groups:
                    nc.gpsimd.memset(
                        group_ap(apair[:, 0], [(t, s) for t, s, _ in run], w,
                                 lead=(NT * W, 2)), 0.0)
                return apair

            def score_tile(qk, apair, j, t):
                """S^T matmuls (row-packed pair) + exp for key tile t."""
                w = widths[t]
                el = elos[t]
                psp = psS_pool.tile([P, 2, N], fp32, tag="psS")
                for hh in range(2):
                    sl = slice(hh * HD, (hh + 1) * HD)
                    nc.tensor.matmul(
                        psp[:, hh, 0:w],
                        lhsT=qk[sl, CT + j, t * P:(t + 1) * P],
                        rhs=qk[sl, j, offs[t] + el:offs[t] + el + w],
                        start=True, stop=True)
                nc.scalar.activation(
                    out=apair[:, :, t, el:el + w], in_=psp[:, :, 0:w],
                    func=AF.Exp)

            def mask_mul(apair, j):
                for hh in range(2):
                    # GPSIMD is ~3x slower per op; give it the first pair so
                    # it finishes long before the next batch's attnV needs it
                    eng = nc.gpsimd if j == 0 else nc.vector
                    for w, run in mask_groups:
                        r = [(t, s) for t, s, _ in run]
                        eng.tensor_mul(
                            group_ap(apair[:, hh], r, w),
                            group_ap(apair[:, hh], r, w),
                            group_ap(m01, r, w))

            def attnv_quarter(apairs, vplus, oc, s, q, state):
                """Quarter q (0..3) of query block s: two heads' attn @ [v|1]
                matmuls into the current 4-head PSUM bank (start=True only on
                the bank's first matmul), plus the bank's normalization when
                its 4 heads are complete."""
                cl = chunks[s]
                if q % 2 == 0:
                    pso = psO_pool.tile([P, 4, P], fp32, tag="psO")
                    state["pso"] = pso
                pso = state["pso"]
                for hh2 in range(2):
                    hh = (q % 2) * 2 + hh2
                    h = (q // 2) * 4 + hh
                    for ci, (t, lo, hi) in enumerate(cl):
                        nc.tensor.matmul(
                            pso[lo - s * P:hi - s * P, hh, 0:HD + 1],
                            lhsT=apairs[h // 2][
                                :, h % 2, t, lo - offs[t]:hi - offs[t]],
                            rhs=vplus[:, t, h, :],
                            start=(hh == 0 and ci == 0),
                            stop=(hh == 3 and ci == len(cl) - 1),
                            skip_group_check=True)
                if q % 2 == 1:
                    g = q // 2
                    rec = rec_pool.tile([P, 4], fp32, tag="rec")
                    nc.vector.reciprocal(rec, pso[:, :, HD])
                    ra = rec[:, :]
                    rec_b = bass.AP(
                        tensor=ra.tensor, offset=ra.offset,
                        ap=[ra.ap[0], [1, 4], [0, HD]])
                    nc.vector.tensor_mul(
                        oc[:, g * C // 2:(g + 1) * C // 2].rearrange(
                            "p (h d) -> p h d", h=4),
                        pso[:, :, 0:HD], rec_b)

            def out_block(oc, b, s):
                """Transpose query-block s of outcat to channel-major,
                project, add bias, stream to DRAM."""
                pst = psB_pool.tile([P, N], bf16, tag="psB")
                for ct in range(CT):
                    nc.tensor.matmul(
                        pst[:, ct * P:(ct + 1) * P],
                        lhsT=oc[:, ct * P:(ct + 1) * P],
                        rhs=ident, is_transpose=True,
                        start=(ct == 0), stop=(ct == CT - 1),
                        skip_group_check=True)
                ocTs = rec_pool.tile([P, CT, P], bf16, tag="ocTs")
                nc.vector.tensor_copy(
                    out=ocTs, in_=pst.rearrange("p (c n) -> p c n", c=CT))
                ps = psB_pool.tile([P, C], fp32, tag="psB")
                for ct in range(CT):
                    nc.tensor.matmul(
                        ps,
                        lhsT=ocTs[:, ct, :],
                        rhs=wproj[:, ct, :],
                        start=(ct == 0), stop=(ct == CT - 1))
                ysb = rec_pool.tile([P, C], fp32, tag="ysb")
                nc.vector.tensor_add(ysb, ps, brep)
                nc.sync.dma_start(
                    out=d_y.ap()[b, s * P:(s + 1) * P, :], in_=ysb)

            # ---- software-pipelined batch loop ----
            qk, vplus = qkv_compute(xts)
            wproj, m01, brep, ident = load_rest()
            apairs = []
            for j in range(CT):
                apair = new_apair()
                apairs.append(apair)
                for t in range(NT):
                    score_tile(qk, apair, j, t)
                mask_mul(apair, j)
            xts_pre = xt_load(1) if BPC > 1 else None
            for b in range(BPC):
                if b + 1 < BPC:
                    qk_n, vplus_n = qkv_compute(xts_pre)
                    xts_pre = xt_load(b + 2) if b + 2 < BPC else None
                else:
                    qk_n = vplus_n = None
                apairs_n = []
                for j in range(CT):
                    # fine interleave: each S key-tile of batch b+1's pair j
                    # alternates with a quarter of batch b's attnV block j;
                    # block j's transposes + projection chain right after.
                    apair_n = None
                    if qk_n is not None:
                        apair_n = new_apair()
                        apairs_n.append(apair_n)
                    oc = oc_pool.tile([P, C], bf16, tag="oc")
                    st = {}
                    for t in range(NT):
                        if apair_n is not None:
                            score_tile(qk_n, apair_n, j, t)
                        attnv_quarter(apairs, vplus, oc, j, t, st)
                    if apair_n is not None:
                        mask_mul(apair_n, j)
                    out_block(oc, b, j)
                qk, vplus, apairs = qk_n, vplus_n, apairs_n

    nc.compile()
    return nc


def _prep(x, w_qkv, w_proj, b_proj, mask):
    x = np.asarray(x, np.float32)
    w_qkv = np.asarray(w_qkv, np.float32)
    w_proj = np.asarray(w_proj, np.float32)
    b_proj = np.asarray(b_proj, np.float32)
    mask2d = np.asarray(mask, np.float32).reshape(N, N)

    W, offs, elos, widths, pads, chunks = _mask_structure(mask2d)

    ws = w_qkv.copy()
    ws[:, :C] *= SCALE  # fold q scaling into the weights
    wqkv_b = ws.astype(_BF16)
    wproj_b = w_proj.astype(_BF16)
    brep = np.tile(b_proj.reshape(1, C), (P, 1)).astype(np.float32)

    vis = (mask2d == 0.0)
    m01 = np.zeros((P, NT, W), np.float32)
    for t in range(NT):
        # m01[p, t, c] = visible(query=offs[t]+c, key=t*128+p)
        hi = min(offs[t] + W, N)
        m01[:, t, 0:hi - offs[t]] = vis[offs[t]:hi, t * P:(t + 1) * P].T
    m01_b = m01.astype(_BF16)

    # xT per core: [NCORES, BPC, C, N]
    xt = np.ascontiguousarray(
        x.reshape(NCORES, BPC, N, C).transpose(0, 1, 3, 2)).astype(_BF16)
    key = (W, tuple(offs), tuple(elos), tuple(widths),
           tuple(pads), tuple(tuple(c) for c in chunks))
    return xt, wqkv_b, wproj_b, brep, m01_b, key


LAST_RESULTS = None


def kernel(x, w_qkv, w_proj, b_proj, mask, _trace=False):
    global LAST_RESULTS
    from concourse import bass_utils

    xt, wqkv_b, wproj_b, brep, m01_b, key = _prep(
        x, w_qkv, w_proj, b_proj, mask)
    W, offs, elos, widths, pads, chunks = key

    if key not in _cache:
        _cache[key] = _build(W, list(offs), list(elos), list(widths),
                             list(pads), [list(c) for c in chunks])
    nc = _cache[key]

    in_maps = []
    for core in range(NCORES):
        in_maps.append({
            "xt": xt[core],
            "wqkv": wqkv_b,
            "wproj": wproj_b,
            "brep": brep,
            "m01": m01_b,
        })
    res = bass_utils.run_bass_kernel_spmd(
        nc, in_maps, core_ids=list(range(NCORES)), trace=_trace)
    LAST_RESULTS = res
    y = np.concatenate([res.results[c]["y"] for c in range(NCORES)], axis=0)
    return y.reshape(B, N, C).astype(np.float32)
